# revision 26
# baseline (speedup 1.0000x reference)
"""Trainium2 Bass kernel for nn_BaichuanAttention (ALiBi attention + KV cache).

Head-parallel across 8 NeuronCores (4 heads/core). Per core:
  - Q/K projection: fp8e4 DoubleRow matmuls (256-deep contraction at
    0.5 cycles/row); weights resident in SBUF. Scores insensitive to
    fp8 rounding (ALiBi dominates the softmax).
  - V projection: 3-term hi/lo fp8 DoubleRow (x = xh + xl, Wv = wh + wl,
    v ~= wh@xh + wh@xl + wl@xh) giving ~fp16 accuracy at 0.75x the fp16
    matmul cost.
  - attention with transposed scores [keys, queries]:
      * ALiBi per-key term folded into the ACT exp() per-partition bias
      * fp8 scale compensation folded into the ACT exp() input scale
        (separate scales for past-key fp16 tiles vs new-key tiles)
      * ALiBi per-query term + causal mask applied via DVE adds
      * softmax denominator via ones-matmul on the PE
  - PV accumulation, normalize via outer-product broadcast, o_proj partial
Host: shard/transpose/cast/scale inputs, sum fp16 o_proj partials.
"""
import os
import sys

import numpy as np

for _p in ("/opt/trn_rl_repo",):
    if os.path.isdir(_p) and _p not in sys.path:
        sys.path.insert(0, _p)

import ml_dtypes
import concourse.bass as bass
import concourse.mybir as mybir
import concourse.tile as tile
from concourse import bacc
from concourse.bass_utils import run_bass_kernel_spmd
from concourse.masks import make_identity

F32 = mybir.dt.float32
F16 = mybir.dt.float16
F8 = mybir.dt.float8e4
NPF8 = ml_dtypes.float8_e4m3

B, S, D, H, HD, HIST = 1, 2048, 4096, 32, 128, 1024
T = HIST + S
NCORES = 8
HPC = H // NCORES          # heads per core
FPC = HPC * HD             # 512 features per core per section
NST = S // 512             # 4 query supertiles
W = 512                    # supertile width
NKT = T // 128             # 24 key tiles
NKC = D // 128             # 32 contraction chunks for QKV
NPAIR = NKC // 2           # 16 DoubleRow contraction pairs
NEG = -1.0e30

SX = 128.0                 # x fp8 scale
SWQ = 2048.0               # Wq (incl 1/sqrt(hd)) fp8 scale
SWK = 128.0                # Wk fp8 scale
SWV = 128.0                # Wv fp8 scale
# new-key scores carry scale SX*SWQ*SX*SWK; past-key scores only SX*SWQ
# (past_key loaded unscaled fp16)
RS_NEW = SX * SWQ * SX * SWK
RS_PAST = SX * SWQ
A_NEW = 1.0 / RS_NEW       # ACT input descale for new-key tiles
A_PAST = 1.0 / RS_PAST     # ACT input descale for past-key tiles
SV_INV = 1.0 / (SX * SWV)  # V psum descale


def _alibi_slopes(n_heads: int) -> np.ndarray:
    def pow2_slopes(m):
        start = 2.0 ** (-(2.0 ** -(np.log2(m) - 3)))
        return start * (start ** np.arange(m))
    if np.log2(n_heads).is_integer():
        return pow2_slopes(n_heads).astype(np.float32)
    m = 2 ** int(np.floor(np.log2(n_heads)))
    base = pow2_slopes(m)
    extra = pow2_slopes(2 * m)[0::2][: n_heads - m]
    return np.concatenate([base, extra]).astype(np.float32)


# --- ALiBi window truncation ---------------------------------------------
# Keys further than MARGIN/slope behind a query contribute exp(-MARGIN)
# relative weight — drop their tiles. Heads are distributed so core c gets
# heads {c, 8+c, 16+c, 24+c} (slot i = head 8i+c); each slot's window is
# sized for the *smallest* slope in its group, so every core runs an
# identical instruction stream.
MARGIN = 18.0
_SLOPES_ALL = _alibi_slopes(H)
JMIN = []
for _i in range(HPC):
    _win = MARGIN / float(_SLOPES_ALL[8 * _i + 7])
    JMIN.append([
        max(0, min(12 + 4 * _s, int((HIST + W * _s - _win) // 128)))
        for _s in range(NST)
    ])


def _emit(nc):
    """Emit the whole per-core program under a TileContext."""
    x8_d = nc.dram_tensor("x8", [128, NPAIR, 2, S], F8, kind="ExternalInput").ap()
    xl8_d = nc.dram_tensor("xl8", [128, NPAIR, 2, S], F8, kind="ExternalInput").ap()
    w8_d = nc.dram_tensor("w8", [NPAIR, 128, 2, 1024], F8, kind="ExternalInput").ap()
    wv8h_d = nc.dram_tensor("wv8h", [NPAIR, 128, 2, FPC], F8, kind="ExternalInput").ap()
    wv8l_d = nc.dram_tensor("wv8l", [NPAIR, 128, 2, FPC], F8, kind="ExternalInput").ap()
    pkT_d = nc.dram_tensor("pkT", [HPC, HD, HIST], F16, kind="ExternalInput").ap()
    pv_d = nc.dram_tensor("pv", [HPC, HIST, HD], F16, kind="ExternalInput").ap()
    opT_d = nc.dram_tensor("opT", [FPC, D], F16, kind="ExternalInput").ap()
    ab_d = nc.dram_tensor("abias", [128, HPC * NST * NKT], F32, kind="ExternalInput").ap()
    dvb_d = nc.dram_tensor("dvb", [HPC, 2, 128, W], F32, kind="ExternalInput").ap()
    mk_d = nc.dram_tensor("mk", [1, 128, 128], F32, kind="ExternalInput").ap()
    out_d = nc.dram_tensor("outp", [S, D], F16, kind="ExternalOutput").ap()

    with tile.TileContext(nc) as tc:
        with (
            tc.tile_pool(name="persist", bufs=1) as pers,
            tc.tile_pool(name="x8slab", bufs=4) as x8pool,
            tc.tile_pool(name="xl8slab", bufs=4) as xl8pool,
            tc.tile_pool(name="qp", bufs=2) as qpool,
            tc.tile_pool(name="opwp", bufs=8) as opwpool,
            tc.tile_pool(name="s1", bufs=3) as s1pool,
            tc.tile_pool(name="pp", bufs=5) as ppool,
            tc.tile_pool(name="dac", bufs=2) as daccpool,
            tc.tile_pool(name="small", bufs=1) as smallpool,
            tc.tile_pool(name="ob", bufs=2) as obpool,
            tc.tile_pool(name="at", bufs=2) as atpool,
            tc.tile_pool(name="ps_qkv", bufs=2, space="PSUM") as ps_qkv,
            tc.tile_pool(name="ps_s", bufs=2, space="PSUM") as ps_s,
            tc.tile_pool(name="ps_o", bufs=2, space="PSUM") as ps_o,
            tc.tile_pool(name="ps_sh", bufs=2, space="PSUM") as ps_sh,
        ):
            # ---- persistent SBUF tensors ----
            kT = [pers.tile([128, S], F16, tag=f"kT{h}", bufs=1, name=f"kT{h}") for h in range(HPC)]
            vT = [pers.tile([128, S], F16, tag=f"vT{h}", bufs=1, name=f"vT{h}") for h in range(HPC)]
            pk_sb = [pers.tile([128, HIST], F16, tag=f"pk{h}", bufs=1, name=f"pk{h}") for h in range(HPC)]
            pv_sb = [pers.tile([128, HIST], F16, tag=f"pvs{h}", bufs=1, name=f"pvs{h}") for h in range(HPC)]
            attn_tiles = {}
            q_tiles = {}
            ab_sb = pers.tile([128, HPC * NST * NKT], F32, tag="abias", bufs=1)
            ident = pers.tile([128, 128], F16, tag="ident", bufs=1)
            ones16 = pers.tile([128, 1], F16, tag="ones16", bufs=1)
            # resident fp8 weights
            w8 = [pers.tile([128, 2, 1024], F8, tag=f"w8_{c}", bufs=1, name=f"w8_{c}")
                  for c in range(NPAIR)]
            wv8h = [pers.tile([128, 2, FPC], F8, tag=f"wvh{c}", bufs=1, name=f"wvh{c}")
                    for c in range(NPAIR)]
            wv8l = [pers.tile([128, 2, FPC], F8, tag=f"wvl{c}", bufs=1, name=f"wvl{c}")
                    for c in range(NPAIR)]

            make_identity(nc, ident)
            nc.any.memset(ones16[:], 1.0)
            nc.gpsimd.dma_start(ab_sb[:], ab_d[:])
            for c in range(NPAIR):
                nc.gpsimd.dma_start(w8[c][:], w8_d[c])
                nc.gpsimd.dma_start(wv8h[c][:], wv8h_d[c])
                nc.gpsimd.dma_start(wv8l[c][:], wv8l_d[c])
            for h in range(HPC):
                nc.gpsimd.dma_start(pk_sb[h][:], pkT_d[h])
                nc.gpsimd.dma_start(
                    pv_sb[h].rearrange("p (c d) -> p c d", c=HIST // 128),
                    pv_d[h].rearrange("(c p) d -> p c d", p=128),
                )

            # per-head row bias [128, W] f32, two pre-scales (past/new keys)
            rowt_p, rowt_n = [], []
            for h in range(HPC):
                tp_ = pers.tile([128, W], F32, tag=f"rowp{h}", bufs=1, name=f"rowp{h}")
                nc.gpsimd.dma_start(tp_[:], dvb_d[h, 0])
                rowt_p.append(tp_)
                tn_ = pers.tile([128, W], F32, tag=f"rown{h}", bufs=1, name=f"rown{h}")
                nc.gpsimd.dma_start(tn_[:], dvb_d[h, 1])
                rowt_n.append(tn_)
            tri = pers.tile([128, 128], F32, tag="tri", bufs=1)
            nc.gpsimd.dma_start(tri[:], mk_d[0])

            def qkv_groups(sc):
                """Return filler closures for supertile sc's QKV projection:
                one x-load group, 8 Q/K feature-chunk groups, 4 V groups."""
                x8t = [None] * 4
                xl8t = [None] * 4

                def xload():
                    for g in range(4):
                        t = x8pool.tile([128, 4, 2, W], F8, tag="x8",
                                        name=f"x8_{sc}_{g}")
                        nc.sync.dma_start(
                            t[:],
                            x8_d[:, g * 4:(g + 1) * 4, :, sc * W:(sc + 1) * W],
                        )
                        x8t[g] = t
                        tl = xl8pool.tile([128, 4, 2, W], F8, tag="xl8",
                                          name=f"xl8_{sc}_{g}")
                        nc.sync.dma_start(
                            tl[:],
                            xl8_d[:, g * 4:(g + 1) * 4, :, sc * W:(sc + 1) * W],
                        )
                        xl8t[g] = tl

                def qk_group(fc):
                    def emit():
                        psum = ps_qkv.tile([128, W], F32, tag="qkvps", name="qkvps")
                        for c in range(NPAIR):
                            nc.tensor.matmul(
                                psum[:],
                                w8[c][:, :, fc * 128:(fc + 1) * 128],
                                x8t[c // 4][:, c % 4, :, :],
                                start=(c == 0), stop=(c == NPAIR - 1),
                                perf_mode=mybir.MatmulPerfMode.DoubleRow,
                            )
                        if fc < 4:
                            qt = qpool.tile([128, W], F16, tag=f"q{fc}",
                                            name=f"q{fc}_{sc}")
                            q_tiles[(fc, sc)] = qt
                            nc.any.tensor_copy(qt[:], psum[:])
                        else:
                            nc.any.tensor_copy(
                                kT[fc - 4][:, sc * W:(sc + 1) * W], psum[:])
                    return emit

                def v_group(fi):
                    def emit():
                        psum = ps_qkv.tile([128, W], F32, tag="qkvps", name="qkvps")
                        nmm = 3 * NPAIR
                        i = 0
                        for c in range(NPAIR):
                            for (wt, xt) in (
                                (wv8h[c], x8t[c // 4][:, c % 4, :, :]),
                                (wv8h[c], xl8t[c // 4][:, c % 4, :, :]),
                                (wv8l[c], x8t[c // 4][:, c % 4, :, :]),
                            ):
                                nc.tensor.matmul(
                                    psum[:],
                                    wt[:, :, fi * 128:(fi + 1) * 128], xt,
                                    start=(i == 0), stop=(i == nmm - 1),
                                    perf_mode=mybir.MatmulPerfMode.DoubleRow,
                                )
                                i += 1
                        nc.vector.tensor_scalar_mul(
                            vT[fi][:, sc * W:(sc + 1) * W], psum[:], SV_INV)
                    return emit

                return ([xload] + [qk_group(fc) for fc in range(8)]
                        + [v_group(fi) for fi in range(4)])

            def vtrans_groups(sc):
                """Transpose new-v chunks for supertile sc in place (4 closures)."""
                def grp(h):
                    def emit():
                        for t4 in range(4):
                            t = sc * 4 + t4
                            tp = ps_sh.tile([128, W], F16, tag="sh", name="vtps")
                            nc.tensor.transpose(
                                tp[:, :128], vT[h][:, t * 128:(t + 1) * 128],
                                ident[:],
                            )
                            nc.any.tensor_copy(
                                vT[h][:, t * 128:(t + 1) * 128], tp[:, :128]
                            )
                    return emit
                return [grp(h) for h in range(HPC)]

            def oproj_groups(s):
                """o_proj partial rows for supertile s (32 closures, opT streamed)."""
                out = []
                for n in range(8):
                    opn = [None] * HPC

                    def wload(n=n, opn=opn):
                        for h in range(HPC):
                            t = opwpool.tile([128, W], F16, tag="opw",
                                             name=f"opw{n}_{h}_{s}")
                            nc.sync.dma_start(
                                t[:],
                                opT_d[h * 128:(h + 1) * 128,
                                      n * W:(n + 1) * W],
                            )
                            opn[h] = t

                    for m4 in range(4):
                        def grp(n=n, m4=m4, opn=opn, wl=(wload if m4 == 0 else None)):
                            if wl is not None:
                                wl()
                            m = s * 4 + m4
                            psum = ps_sh.tile([128, W], F32, tag="sh",
                                              name="shps")
                            for h in range(HPC):
                                nc.tensor.matmul(
                                    psum[:],
                                    attn_tiles[(h, s)][:, m4 * 128:(m4 + 1) * 128],
                                    opn[h][:],
                                    start=(h == 0), stop=(h == HPC - 1),
                                )
                            ob = obpool.tile([128, W], F16, tag="ob", name="ob")
                            nc.any.tensor_copy(ob[:], psum[:])
                            nc.sync.dma_start(
                                out_d[m * 128:(m + 1) * 128,
                                      n * W:(n + 1) * W], ob[:]
                            )
                        out.append(grp)
                return out

            def attention_all(s, fillers):
                """All heads for supertile s, software-pipelined (scores run
                DEPTH tiles ahead of PV) with filler groups interleaved to
                keep the PE busy during the add->exp latency chain."""
                nvis = 12 + 4 * s
                tiles = [(h, j) for h in range(HPC)
                         for j in range(JMIN[h][s], nvis)]
                DEPTH = 2 if s <= 1 else 4
                ntiles = len(tiles)
                nfill = len(fillers)
                filled = 0
                state = {}   # h -> (o_ps, acc)
                pend = []    # [(h, j, p, off, nv)]
                scnt = 0

                def emit_scores(h, j):
                    nonlocal scnt
                    m = j - (8 + 4 * s)
                    off = 0 if m < 0 else 128 * m
                    nv = W - off
                    if s >= 2 and scnt % 2 == 1:
                        sp = ps_qkv.tile([128, W], F32, tag="qkvps", name="qkvps")
                    else:
                        sp = ps_s.tile([128, W], F32, tag="sps", name="sps")
                    scnt += 1
                    if j < 8:
                        kt = pk_sb[h][:, j * 128:(j + 1) * 128]
                    else:
                        kt = kT[h][:, (j - 8) * 128:(j - 7) * 128]
                    nc.tensor.matmul(
                        sp[:, :nv], kt,
                        q_tiles[(h, s)][:, off:],
                        start=True, stop=True,
                    )
                    rowt = rowt_p[h] if j < 8 else rowt_n[h]
                    s1 = s1pool.tile([128, W], F32, tag="s1", name="s1")
                    nc.vector.tensor_add(s1[:, :nv], sp[:, :nv], rowt[:, off:])
                    if m >= 0:
                        nc.vector.tensor_add(s1[:, :128], s1[:, :128], tri[:])
                    p = ppool.tile([128, W], F16, tag="p", name="p")
                    col = (h * NST + s) * NKT + j
                    nc.scalar.activation(
                        p[:, :nv], s1[:, :nv],
                        mybir.ActivationFunctionType.Exp,
                        bias=ab_sb[:, col:col + 1],
                        scale=(A_PAST if j < 8 else A_NEW),
                    )
                    pend.append((h, j, p, off, nv))

                def emit_pv():
                    h, j, p, off, nv = pend.pop(0)
                    j0 = JMIN[h][s]
                    if j == j0:
                        o_ps = ps_o.tile([128, W], F32, tag="ops",
                                         name=f"ops{h}")
                        acc = daccpool.tile([128, W], F16, tag="dacc",
                                            name=f"dacc{h}_{s}")
                        state[h] = (o_ps, acc)
                    o_ps, acc = state[h]
                    if j < 8:
                        vt = pv_sb[h][:, j * 128:(j + 1) * 128]
                    else:
                        vt = vT[h][:, (j - 8) * 128:(j - 7) * 128]
                    nc.tensor.matmul(
                        o_ps[:, off:], vt, p[:, :nv],
                        start=(j == j0), stop=(j == nvis - 1),
                    )
                    # denominator accumulation on DVE (fp16, 2x mode)
                    if j == j0:
                        if off:
                            nc.vector.memset(acc[:, :off], 0.0)
                        nc.vector.tensor_copy(acc[:, off:], p[:, :nv])
                    else:
                        nc.vector.tensor_add(
                            acc[:, off:], acc[:, off:], p[:, :nv])
                    if j == nvis - 1:
                        # denominator + normalize (d reuses a scores psum slot)
                        d_ps = ps_s.tile([128, W], F32, tag="sps", name="dps")
                        nc.tensor.matmul(
                            d_ps[0:1, :], ones16[:], acc[:],
                            start=True, stop=True,
                        )
                        denr = smallpool.tile([1, W], F32, tag="denr",
                                              name="denr")
                        nc.vector.reciprocal(denr[:], d_ps[0:1, :])
                        bb = s1pool.tile([128, W], F32, tag="bb", bufs=2,
                                         name="bb")
                        nc.gpsimd.partition_broadcast(bb[:], denr[:])
                        at = atpool.tile([128, W], F16, tag=f"at{h}",
                                         name=f"at{h}_{s}")
                        attn_tiles[(h, s)] = at
                        nc.vector.tensor_mul(at[:], o_ps[:], bb[:])

                for idx, (h, j) in enumerate(tiles):
                    emit_scores(h, j)
                    # interleave filler work proportionally
                    want = (idx + 1) * nfill // ntiles
                    while filled < want:
                        fillers[filled]()
                        filled += 1
                    if len(pend) >= DEPTH:
                        emit_pv()
                while pend:
                    emit_pv()
                while filled < nfill:
                    fillers[filled]()
                    filled += 1

            # ---- pipelined emission order ----
            for g in qkv_groups(0):
                g()
            for g in qkv_groups(1):
                g()
            for g in vtrans_groups(0):
                g()
            for g in vtrans_groups(1):
                g()
            attention_all(0, qkv_groups(2) + vtrans_groups(2))
            attention_all(1, qkv_groups(3) + vtrans_groups(3) + oproj_groups(0))
            attention_all(2, oproj_groups(1))
            attention_all(3, oproj_groups(2))
            for g in oproj_groups(3):
                g()

    return nc


_CACHE = {}


def _build():
    if "nc" not in _CACHE:
        nc = bacc.Bacc(
            trn_type="TRN2", target_bir_lowering=False, debug=False,
            num_devices=NCORES,
        )
        _emit(nc)
        nc.compile()
        _CACHE["nc"] = nc
    return _CACHE["nc"]


def _pair8(a):
    """[D, F] -> fp8 pair layout [NPAIR, 128, 2, F]."""
    Dd, F = a.shape
    return np.ascontiguousarray(
        a.reshape(NPAIR, 2, 128, F).transpose(0, 2, 1, 3)
    ).astype(NPF8)


def _pair8_pm(a):
    """[D, F] -> fp8 partition-major pair layout [128, NPAIR, 2, F]."""
    Dd, F = a.shape
    return np.ascontiguousarray(
        a.reshape(NPAIR, 2, 128, F).transpose(2, 0, 1, 3)
    ).astype(NPF8)


def _host_prep(hidden_states, past_key, past_value, W_pack_w, o_proj_w):
    x = np.asarray(hidden_states, np.float32).reshape(S, D)
    pk = np.asarray(past_key, np.float32).reshape(H, HIST, HD)
    pv = np.asarray(past_value, np.float32).reshape(H, HIST, HD)
    Wp = np.asarray(W_pack_w, np.float32)
    Wo = np.asarray(o_proj_w, np.float32)
    slopes = _alibi_slopes(H)

    xT = np.ascontiguousarray(x.T)
    xs = xT * SX
    xh = xs.astype(NPF8).astype(np.float32)
    x8 = _pair8_pm(xh)                              # hi (exactly representable)
    xl8 = _pair8_pm(xs - xh)                        # lo residual

    scale = np.float32(1.0 / np.sqrt(HD))
    kk = np.arange(128, dtype=np.float32)
    ii = np.arange(W, dtype=np.float32)

    in_maps = []
    for c in range(NCORES):
        heads = [8 * i + c for i in range(HPC)]
        rsel = np.concatenate(
            [np.arange(hh * HD, (hh + 1) * HD) for hh in heads])
        Wq = Wp[rsel] * scale
        Wk = Wp[D + rsel]
        Wv = Wp[2 * D + rsel]
        Wqk = np.concatenate([Wq * SWQ, Wk * SWK], 0).T  # [D, 1024]
        w8 = _pair8(Wqk)
        wvs = Wv.T * SWV                                 # [D, FPC]
        wvh = wvs.astype(NPF8).astype(np.float32)
        wv8h = _pair8(wvh)
        wv8l = _pair8(wvs - wvh)
        pkT = np.ascontiguousarray(
            pk[heads].transpose(0, 2, 1)
        ).astype(np.float16)
        pvc = np.ascontiguousarray(pv[heads]).astype(np.float16)
        opT = np.ascontiguousarray(
            Wo[:, rsel].T
        ).astype(np.float16)
        sl = slopes[heads]

        ab = np.zeros((HPC, NST, NKT, 128), np.float32)
        for h in range(HPC):
            for s in range(NST):
                for j in range(NKT):
                    ab[h, s, j] = sl[h] * (128 * j + kk - HIST - W * s)
        ab_sb = np.ascontiguousarray(
            ab.reshape(HPC * NST * NKT, 128).T
        )

        dvb = np.zeros((HPC, 2, 128, W), np.float32)
        for h in range(HPC):
            dvb[h, 0] = (np.float32(RS_PAST) * -sl[h] * ii)[None, :]
            dvb[h, 1] = (np.float32(RS_NEW) * -sl[h] * ii)[None, :]
        mk = np.where(ii[None, None, :128] >= kk[None, :, None], 0.0,
                      NEG).astype(np.float32)
        in_maps.append({
            "x8": x8, "xl8": xl8, "w8": w8, "wv8h": wv8h, "wv8l": wv8l,
            "pkT": pkT, "pv": pvc, "opT": opT, "abias": ab_sb,
            "dvb": dvb, "mk": mk,
        })
    return in_maps


def kernel(hidden_states, past_key, past_value, W_pack_w, o_proj_w):
    nc = _build()
    in_maps = _host_prep(hidden_states, past_key, past_value, W_pack_w, o_proj_w)
    res = run_bass_kernel_spmd(nc, in_maps, list(range(NCORES)))
    out = np.zeros((S, D), np.float64)
    for c in range(NCORES):
        out += res.results[c]["outp"].astype(np.float64)
    return out.astype(np.float32).reshape(B, S, D)


# revision 38
# speedup vs baseline: 1.1850x; 1.1850x over previous
"""Trainium2 Bass kernel for nn_BaichuanAttention (ALiBi attention + KV cache).

Head-parallel across 8 NeuronCores (4 heads/core). Per core:
  - Q/K projection: fp8e4 DoubleRow matmuls (256-deep contraction at
    0.5 cycles/row); weights resident in SBUF. Scores insensitive to
    fp8 rounding (ALiBi dominates the softmax).
  - V projection: 3-term hi/lo fp8 DoubleRow (x = xh + xl, Wv = wh + wl,
    v ~= wh@xh + wh@xl + wl@xh) giving ~fp16 accuracy at 0.75x the fp16
    matmul cost.
  - attention with transposed scores [keys, queries]:
      * ALiBi per-key term folded into the ACT exp() per-partition bias
      * fp8 scale compensation folded into the ACT exp() input scale
        (separate scales for past-key fp16 tiles vs new-key tiles)
      * ALiBi per-query term + causal mask applied via DVE adds
      * softmax denominator via ones-matmul on the PE
  - PV accumulation, normalize via outer-product broadcast, o_proj partial
Host: shard/transpose/cast/scale inputs, sum fp16 o_proj partials.
"""
import os
import sys

import numpy as np

for _p in ("/opt/trn_rl_repo",):
    if os.path.isdir(_p) and _p not in sys.path:
        sys.path.insert(0, _p)

import ml_dtypes
import concourse.bass as bass
import concourse.mybir as mybir
import concourse.tile as tile
from concourse import bacc
from concourse.bass_utils import run_bass_kernel_spmd
from concourse.masks import make_identity

F32 = mybir.dt.float32
F16 = mybir.dt.float16
F8 = mybir.dt.float8e4
NPF8 = ml_dtypes.float8_e4m3

B, S, D, H, HD, HIST = 1, 2048, 4096, 32, 128, 1024
T = HIST + S
NCORES = 8
HPC = H // NCORES          # heads per core
FPC = HPC * HD             # 512 features per core per section
NST = S // 512             # 4 query supertiles
W = 512                    # supertile width
NKT = T // 128             # 24 key tiles
NKC = D // 128             # 32 contraction chunks for QKV
NPAIR = NKC // 2           # 16 DoubleRow contraction pairs
NEG = -1.0e30

SX = 128.0                 # x fp8 scale
SWQ = 2048.0               # Wq (incl 1/sqrt(hd)) fp8 scale
SWK = 128.0                # Wk fp8 scale
SWV = 128.0                # Wv fp8 scale
# new-key scores carry scale SX*SWQ*SX*SWK; past-key scores only SX*SWQ
# (past_key loaded unscaled fp16)
RS_NEW = SX * SWQ * SX * SWK
RS_PAST = SX * SWQ
A_NEW = 1.0 / RS_NEW       # ACT input descale for new-key tiles
A_PAST = 1.0 / RS_PAST     # ACT input descale for past-key tiles
SV_INV = 1.0 / (SX * SWV)  # V psum descale


def _alibi_slopes(n_heads: int) -> np.ndarray:
    def pow2_slopes(m):
        start = 2.0 ** (-(2.0 ** -(np.log2(m) - 3)))
        return start * (start ** np.arange(m))
    if np.log2(n_heads).is_integer():
        return pow2_slopes(n_heads).astype(np.float32)
    m = 2 ** int(np.floor(np.log2(n_heads)))
    base = pow2_slopes(m)
    extra = pow2_slopes(2 * m)[0::2][: n_heads - m]
    return np.concatenate([base, extra]).astype(np.float32)


# --- ALiBi window truncation ---------------------------------------------
# Keys further than MARGIN/slope behind a query contribute exp(-MARGIN)
# relative weight — drop their tiles. Heads are distributed so core c gets
# heads {c, 8+c, 16+c, 24+c} (slot i = head 8i+c); each slot's window is
# sized for the *smallest* slope in its group, so every core runs an
# identical instruction stream.
MARGIN = 18.0
_SLOPES_ALL = _alibi_slopes(H)
JMIN = []
for _i in range(HPC):
    _win = MARGIN / float(_SLOPES_ALL[8 * _i + 7])
    JMIN.append([
        max(0, min(12 + 4 * _s, int((HIST + W * _s - _win) // 128)))
        for _s in range(NST)
    ])
# past-key/value tiles actually reachable per slot (j in [PK0[h], 8))
PK0 = [min(JMIN[_i][0], 8) for _i in range(HPC)]
NPK = [8 - PK0[_i] for _i in range(HPC)]
PKOFF = [sum(NPK[:_i]) for _i in range(HPC)]   # segment offsets, in tiles
NPKT = sum(NPK)


def _emit(nc):
    """Emit the whole per-core program under a TileContext."""
    x8_d = nc.dram_tensor("x8", [128, NPAIR, 2, S], F8, kind="ExternalInput").ap()
    xl8_d = nc.dram_tensor("xl8", [128, NPAIR, 2, S], F8, kind="ExternalInput").ap()
    w8_d = nc.dram_tensor("w8", [NPAIR, 128, 2, 1024], F8, kind="ExternalInput").ap()
    wv8h_d = nc.dram_tensor("wv8h", [NPAIR, 128, 2, FPC], F8, kind="ExternalInput").ap()
    wv8l_d = nc.dram_tensor("wv8l", [NPAIR, 128, 2, FPC], F8, kind="ExternalInput").ap()
    pkT_d = nc.dram_tensor("pkT", [HD, NPKT * 128], F16, kind="ExternalInput").ap()
    pv_d = nc.dram_tensor("pv", [NPKT * 128, HD], F16, kind="ExternalInput").ap()
    opT_d = nc.dram_tensor("opT", [FPC, D], F16, kind="ExternalInput").ap()
    ab_d = nc.dram_tensor("abias", [128, HPC * NST * NKT], F32, kind="ExternalInput").ap()
    dvb_d = nc.dram_tensor("dvb", [HPC, 2, 128, W], F32, kind="ExternalInput").ap()
    mk_d = nc.dram_tensor("mk", [1, 128, 128], F32, kind="ExternalInput").ap()
    out_d = nc.dram_tensor("outp", [S, D], F16, kind="ExternalOutput").ap()

    with tile.TileContext(nc) as tc:
        with (
            tc.tile_pool(name="persist", bufs=1) as pers,
            tc.tile_pool(name="x8slab", bufs=2) as x8pool,
            tc.tile_pool(name="xl8slab", bufs=2) as xl8pool,
            tc.tile_pool(name="qp", bufs=2) as qpool,
            tc.tile_pool(name="opwp", bufs=2) as opwpool,
            tc.tile_pool(name="s1", bufs=3) as s1pool,
            tc.tile_pool(name="pp", bufs=5) as ppool,
            tc.tile_pool(name="dac", bufs=2) as daccpool,
            tc.tile_pool(name="small", bufs=1) as smallpool,
            tc.tile_pool(name="ob", bufs=2) as obpool,
            tc.tile_pool(name="at", bufs=2) as atpool,
            tc.tile_pool(name="ps_qkv", bufs=2, space="PSUM") as ps_qkv,
            tc.tile_pool(name="ps_s", bufs=2, space="PSUM") as ps_s,
            tc.tile_pool(name="ps_o", bufs=2, space="PSUM") as ps_o,
            tc.tile_pool(name="ps_sh", bufs=2, space="PSUM") as ps_sh,
        ):
            # ---- persistent SBUF tensors ----
            kT = [pers.tile([128, S], F16, tag=f"kT{h}", bufs=1, name=f"kT{h}") for h in range(HPC)]
            vT = [pers.tile([128, S], F16, tag=f"vT{h}", bufs=1, name=f"vT{h}") for h in range(HPC)]
            pk_sb = [pers.tile([128, NPK[h] * 128], F16, tag=f"pk{h}", bufs=1, name=f"pk{h}") for h in range(HPC)]
            pv_sb = [pers.tile([128, NPK[h] * 128], F16, tag=f"pvs{h}", bufs=1, name=f"pvs{h}") for h in range(HPC)]
            attn_tiles = {}
            q_tiles = {}
            ab_sb = pers.tile([128, HPC * NST * NKT], F32, tag="abias", bufs=1)
            ident = pers.tile([128, 128], F16, tag="ident", bufs=1)
            ones16 = pers.tile([128, 1], F16, tag="ones16", bufs=1)
            # resident fp8 weights
            w8 = [pers.tile([128, 2, 1024], F8, tag=f"w8_{c}", bufs=1, name=f"w8_{c}")
                  for c in range(NPAIR)]
            wv8h = [pers.tile([128, 2, FPC], F8, tag=f"wvh{c}", bufs=1, name=f"wvh{c}")
                    for c in range(NPAIR)]
            wv8l = [pers.tile([128, 2, FPC], F8, tag=f"wvl{c}", bufs=1, name=f"wvl{c}")
                    for c in range(NPAIR)]

            make_identity(nc, ident)
            nc.any.memset(ones16[:], 1.0)
            # init loads spread across DMA-issue queues: Pool takes the
            # critical w8, ACT takes wv8h, DVE takes wv8l + small tables.
            # SP (sync) is reserved for x8 streaming + output stores.
            for c in range(NPAIR):
                nc.gpsimd.dma_start(w8[c][:], w8_d[c])
            for c in range(NPAIR):
                nc.scalar.dma_start(wv8h[c][:], wv8h_d[c])
                nc.scalar.dma_start(wv8l[c][:], wv8l_d[c])
            nc.scalar.dma_start(ab_sb[:], ab_d[:])
            for h in range(HPC):
                nc.scalar.dma_start(
                    pk_sb[h][:],
                    pkT_d[:, PKOFF[h] * 128:(PKOFF[h] + NPK[h]) * 128])
                nc.scalar.dma_start(
                    pv_sb[h].rearrange("p (c d) -> p c d", c=NPK[h]),
                    pv_d[PKOFF[h] * 128:(PKOFF[h] + NPK[h]) * 128]
                    .rearrange("(c p) d -> p c d", p=128),
                )

            # per-head row bias [128, W] f32, two pre-scales (past/new keys)
            rowt_p, rowt_n = [], []
            for h in range(HPC):
                tp_ = pers.tile([128, W], F32, tag=f"rowp{h}", bufs=1, name=f"rowp{h}")
                nc.scalar.dma_start(tp_[:], dvb_d[h, 0])
                rowt_p.append(tp_)
                tn_ = pers.tile([128, W], F32, tag=f"rown{h}", bufs=1, name=f"rown{h}")
                nc.scalar.dma_start(tn_[:], dvb_d[h, 1])
                rowt_n.append(tn_)
            tri = pers.tile([128, 128], F32, tag="tri", bufs=1)
            nc.scalar.dma_start(tri[:], mk_d[0])

            def qkv_groups(sc):
                """Return filler closures for supertile sc's QKV projection:
                one x-load group, 8 Q/K feature-chunk groups, 4 V groups."""
                x8t = [None] * 2
                xl8t = [None] * 2

                def xload():
                    for g in range(2):
                        t = x8pool.tile([128, 8, 2, W], F8, tag="x8",
                                        name=f"x8_{sc}_{g}")
                        nc.sync.dma_start(
                            t[:],
                            x8_d[:, g * 8:(g + 1) * 8, :, sc * W:(sc + 1) * W],
                        )
                        x8t[g] = t
                        tl = xl8pool.tile([128, 8, 2, W], F8, tag="xl8",
                                          name=f"xl8_{sc}_{g}")
                        nc.gpsimd.dma_start(
                            tl[:],
                            xl8_d[:, g * 8:(g + 1) * 8, :, sc * W:(sc + 1) * W],
                        )
                        xl8t[g] = tl

                def qk_group(fc):
                    def emit():
                        psum = ps_qkv.tile([128, W], F32, tag="qkvps", name="qkvps")
                        for c in range(NPAIR):
                            nc.tensor.matmul(
                                psum[:],
                                w8[c][:, :, fc * 128:(fc + 1) * 128],
                                x8t[c // 8][:, c % 8, :, :],
                                start=(c == 0), stop=(c == NPAIR - 1),
                                perf_mode=mybir.MatmulPerfMode.DoubleRow,
                            )
                        if fc < 4:
                            qt = qpool.tile([128, W], F16, tag=f"q{fc}",
                                            name=f"q{fc}_{sc}")
                            q_tiles[(fc, sc)] = qt
                            nc.any.tensor_copy(qt[:], psum[:])
                        else:
                            nc.any.tensor_copy(
                                kT[fc - 4][:, sc * W:(sc + 1) * W], psum[:])
                    return emit

                def v_group(fi):
                    def emit():
                        psum = ps_qkv.tile([128, W], F32, tag="qkvps", name="qkvps")
                        nmm = 3 * NPAIR
                        i = 0
                        for c in range(NPAIR):
                            for (wt, xt) in (
                                (wv8h[c], x8t[c // 8][:, c % 8, :, :]),
                                (wv8h[c], xl8t[c // 8][:, c % 8, :, :]),
                                (wv8l[c], x8t[c // 8][:, c % 8, :, :]),
                            ):
                                nc.tensor.matmul(
                                    psum[:],
                                    wt[:, :, fi * 128:(fi + 1) * 128], xt,
                                    start=(i == 0), stop=(i == nmm - 1),
                                    perf_mode=mybir.MatmulPerfMode.DoubleRow,
                                )
                                i += 1
                        nc.vector.tensor_scalar_mul(
                            vT[fi][:, sc * W:(sc + 1) * W], psum[:], SV_INV)
                    return emit

                return ([xload] + [qk_group(fc) for fc in range(8)]
                        + [v_group(fi) for fi in range(4)])

            def vtrans_groups(sc):
                """Transpose new-v chunks for supertile sc in place (4 closures)."""
                def grp(h):
                    def emit():
                        for t4 in range(4):
                            t = sc * 4 + t4
                            tp = ps_sh.tile([128, W], F16, tag="sh", name="vtps")
                            nc.tensor.transpose(
                                tp[:, :128], vT[h][:, t * 128:(t + 1) * 128],
                                ident[:],
                            )
                            nc.any.tensor_copy(
                                vT[h][:, t * 128:(t + 1) * 128], tp[:, :128]
                            )
                    return emit
                return [grp(h) for h in range(HPC)]

            def oproj_groups(s):
                """o_proj partial rows for supertile s (32 closures; batched
                weight loads on the ACT queue, 2-batched output stores)."""
                out = []
                for n in range(8):
                    opn = [None]
                    obt = [None]

                    def wload(n=n, opn=opn):
                        t = opwpool.tile([128, HPC, W], F16, tag="opw",
                                         name=f"opw{n}_{s}")
                        nc.scalar.dma_start(
                            t[:],
                            opT_d[:, n * W:(n + 1) * W]
                            .rearrange("(hh p) f -> p hh f", p=128),
                        )
                        opn[0] = t

                    for m4 in range(4):
                        def grp(n=n, m4=m4, opn=opn, obt=obt,
                                wl=(wload if m4 == 0 else None)):
                            if wl is not None:
                                wl()
                            m = s * 4 + m4
                            psum = ps_sh.tile([128, W], F32, tag="sh",
                                              name="shps")
                            for h in range(HPC):
                                nc.tensor.matmul(
                                    psum[:],
                                    attn_tiles[(h, s)][:, m4 * 128:(m4 + 1) * 128],
                                    opn[0][:, h, :],
                                    start=(h == 0), stop=(h == HPC - 1),
                                )
                            if m4 % 2 == 0:
                                obt[0] = obpool.tile([128, 2, W], F16,
                                                     tag="ob", name="ob")
                            nc.any.tensor_copy(obt[0][:, m4 % 2, :], psum[:])
                            if m4 % 2 == 1:
                                nc.sync.dma_start(
                                    out_d[(m - 1) * 128:(m + 1) * 128,
                                          n * W:(n + 1) * W]
                                    .rearrange("(two p) f -> p two f", p=128),
                                    obt[0][:],
                                )
                        out.append(grp)
                return out

            def attention_all(s, fillers):
                """All heads for supertile s, software-pipelined (scores run
                DEPTH tiles ahead of PV) with filler groups interleaved to
                keep the PE busy during the add->exp latency chain."""
                nvis = 12 + 4 * s
                tiles = [(h, j) for h in range(HPC)
                         for j in range(JMIN[h][s], nvis)]
                DEPTH = 2 if s <= 1 else 4
                ntiles = len(tiles)
                nfill = len(fillers)
                filled = 0
                state = {}   # h -> (o_ps, acc)
                pend = []    # [(h, j, p, off, nv)]
                scnt = 0

                def emit_scores(h, j):
                    nonlocal scnt
                    m = j - (8 + 4 * s)
                    off = 0 if m < 0 else 128 * m
                    nv = W - off
                    if s >= 2 and scnt % 2 == 1:
                        sp = ps_qkv.tile([128, W], F32, tag="qkvps", name="qkvps")
                    else:
                        sp = ps_s.tile([128, W], F32, tag="sps", name="sps")
                    scnt += 1
                    if j < 8:
                        jj = j - PK0[h]
                        kt = pk_sb[h][:, jj * 128:(jj + 1) * 128]
                    else:
                        kt = kT[h][:, (j - 8) * 128:(j - 7) * 128]
                    nc.tensor.matmul(
                        sp[:, :nv], kt,
                        q_tiles[(h, s)][:, off:],
                        start=True, stop=True,
                    )
                    rowt = rowt_p[h] if j < 8 else rowt_n[h]
                    s1 = s1pool.tile([128, W], F32, tag="s1", name="s1")
                    nc.vector.tensor_add(s1[:, :nv], sp[:, :nv], rowt[:, off:])
                    if m >= 0:
                        nc.vector.tensor_add(s1[:, :128], s1[:, :128], tri[:])
                    p = ppool.tile([128, W], F16, tag="p", name="p")
                    col = (h * NST + s) * NKT + j
                    nc.scalar.activation(
                        p[:, :nv], s1[:, :nv],
                        mybir.ActivationFunctionType.Exp,
                        bias=ab_sb[:, col:col + 1],
                        scale=(A_PAST if j < 8 else A_NEW),
                    )
                    pend.append((h, j, p, off, nv))

                def emit_pv():
                    h, j, p, off, nv = pend.pop(0)
                    j0 = JMIN[h][s]
                    if j == j0:
                        o_ps = ps_o.tile([128, W], F32, tag="ops",
                                         name=f"ops{h}")
                        acc = daccpool.tile([128, W], F16, tag="dacc",
                                            name=f"dacc{h}_{s}")
                        state[h] = (o_ps, acc)
                    o_ps, acc = state[h]
                    if j < 8:
                        jj = j - PK0[h]
                        vt = pv_sb[h][:, jj * 128:(jj + 1) * 128]
                    else:
                        vt = vT[h][:, (j - 8) * 128:(j - 7) * 128]
                    nc.tensor.matmul(
                        o_ps[:, off:], vt, p[:, :nv],
                        start=(j == j0), stop=(j == nvis - 1),
                    )
                    # denominator accumulation on DVE (fp16, 2x mode)
                    if j == j0:
                        if off:
                            nc.vector.memset(acc[:, :off], 0.0)
                        nc.vector.tensor_copy(acc[:, off:], p[:, :nv])
                    else:
                        nc.vector.tensor_add(
                            acc[:, off:], acc[:, off:], p[:, :nv])
                    if j == nvis - 1:
                        # denominator + normalize (d reuses a scores psum slot)
                        d_ps = ps_s.tile([128, W], F32, tag="sps", name="dps")
                        nc.tensor.matmul(
                            d_ps[0:1, :], ones16[:], acc[:],
                            start=True, stop=True,
                        )
                        denr = smallpool.tile([1, W], F32, tag="denr",
                                              name="denr")
                        nc.vector.reciprocal(denr[:], d_ps[0:1, :])
                        bb = s1pool.tile([128, W], F32, tag="bb", bufs=2,
                                         name="bb")
                        nc.gpsimd.partition_broadcast(bb[:], denr[:])
                        at = atpool.tile([128, W], F16, tag=f"at{h}",
                                         name=f"at{h}_{s}")
                        attn_tiles[(h, s)] = at
                        nc.vector.tensor_mul(at[:], o_ps[:], bb[:])

                for idx, (h, j) in enumerate(tiles):
                    emit_scores(h, j)
                    # interleave filler work proportionally
                    want = (idx + 1) * nfill // ntiles
                    while filled < want:
                        fillers[filled]()
                        filled += 1
                    if len(pend) >= DEPTH:
                        emit_pv()
                while pend:
                    emit_pv()
                while filled < nfill:
                    fillers[filled]()
                    filled += 1

            # ---- pipelined emission order ----
            for g in qkv_groups(0):
                g()
            for g in qkv_groups(1):
                g()
            for g in vtrans_groups(0):
                g()
            for g in vtrans_groups(1):
                g()
            attention_all(0, qkv_groups(2) + vtrans_groups(2))
            attention_all(1, qkv_groups(3) + vtrans_groups(3) + oproj_groups(0))
            attention_all(2, oproj_groups(1))
            attention_all(3, oproj_groups(2))
            for g in oproj_groups(3):
                g()

    return nc


_CACHE = {}


def _build():
    if "nc" not in _CACHE:
        nc = bacc.Bacc(
            trn_type="TRN2", target_bir_lowering=False, debug=False,
            num_devices=NCORES,
        )
        _emit(nc)
        nc.compile()
        _CACHE["nc"] = nc
    return _CACHE["nc"]


def _pair8(a):
    """[D, F] -> fp8 pair layout [NPAIR, 128, 2, F]."""
    Dd, F = a.shape
    return np.ascontiguousarray(
        a.reshape(NPAIR, 2, 128, F).transpose(0, 2, 1, 3)
    ).astype(NPF8)


def _pair8_pm(a):
    """[D, F] -> fp8 partition-major pair layout [128, NPAIR, 2, F]."""
    Dd, F = a.shape
    return np.ascontiguousarray(
        a.reshape(NPAIR, 2, 128, F).transpose(2, 0, 1, 3)
    ).astype(NPF8)


def _host_prep(hidden_states, past_key, past_value, W_pack_w, o_proj_w):
    x = np.asarray(hidden_states, np.float32).reshape(S, D)
    pk = np.asarray(past_key, np.float32).reshape(H, HIST, HD)
    pv = np.asarray(past_value, np.float32).reshape(H, HIST, HD)
    Wp = np.asarray(W_pack_w, np.float32)
    Wo = np.asarray(o_proj_w, np.float32)
    slopes = _alibi_slopes(H)

    xT = np.ascontiguousarray(x.T)
    xs = xT * SX
    xh = xs.astype(NPF8).astype(np.float32)
    x8 = _pair8_pm(xh)                              # hi (exactly representable)
    xl8 = _pair8_pm(xs - xh)                        # lo residual

    scale = np.float32(1.0 / np.sqrt(HD))
    kk = np.arange(128, dtype=np.float32)
    ii = np.arange(W, dtype=np.float32)

    in_maps = []
    for c in range(NCORES):
        heads = [8 * i + c for i in range(HPC)]
        rsel = np.concatenate(
            [np.arange(hh * HD, (hh + 1) * HD) for hh in heads])
        Wq = Wp[rsel] * scale
        Wk = Wp[D + rsel]
        Wv = Wp[2 * D + rsel]
        Wqk = np.concatenate([Wq * SWQ, Wk * SWK], 0).T  # [D, 1024]
        w8 = _pair8(Wqk)
        wvs = Wv.T * SWV                                 # [D, FPC]
        wvh = wvs.astype(NPF8).astype(np.float32)
        wv8h = _pair8(wvh)
        wv8l = _pair8(wvs - wvh)
        # flat truncated past-KV: per slot h only tiles [PK0[h], 8)
        pkT = np.concatenate(
            [pk[heads[h]][PK0[h] * 128:, :].T for h in range(HPC)], axis=1
        ).astype(np.float16)
        pkT = np.ascontiguousarray(pkT)
        pvc = np.ascontiguousarray(np.concatenate(
            [pv[heads[h]][PK0[h] * 128:, :] for h in range(HPC)], axis=0
        )).astype(np.float16)
        opT = np.ascontiguousarray(
            Wo[:, rsel].T
        ).astype(np.float16)
        sl = slopes[heads]

        ab = np.zeros((HPC, NST, NKT, 128), np.float32)
        for h in range(HPC):
            for s in range(NST):
                for j in range(NKT):
                    ab[h, s, j] = sl[h] * (128 * j + kk - HIST - W * s)
        ab_sb = np.ascontiguousarray(
            ab.reshape(HPC * NST * NKT, 128).T
        )

        dvb = np.zeros((HPC, 2, 128, W), np.float32)
        for h in range(HPC):
            dvb[h, 0] = (np.float32(RS_PAST) * -sl[h] * ii)[None, :]
            dvb[h, 1] = (np.float32(RS_NEW) * -sl[h] * ii)[None, :]
        mk = np.where(ii[None, None, :128] >= kk[None, :, None], 0.0,
                      NEG).astype(np.float32)
        in_maps.append({
            "x8": x8, "xl8": xl8, "w8": w8, "wv8h": wv8h, "wv8l": wv8l,
            "pkT": pkT, "pv": pvc, "opT": opT, "abias": ab_sb,
            "dvb": dvb, "mk": mk,
        })
    return in_maps


def kernel(hidden_states, past_key, past_value, W_pack_w, o_proj_w):
    nc = _build()
    in_maps = _host_prep(hidden_states, past_key, past_value, W_pack_w, o_proj_w)
    res = run_bass_kernel_spmd(nc, in_maps, list(range(NCORES)))
    out = np.zeros((S, D), np.float64)
    for c in range(NCORES):
        out += res.results[c]["outp"].astype(np.float64)
    return out.astype(np.float32).reshape(B, S, D)
